# revision 10
# baseline (speedup 1.0000x reference)
"""v13.3: data-parallel attention (1 batch element per core, no collectives).

vs v11 baseline (295,868 ns -> ~242,000 ns best measured):
- Weights/constants loaded once and kept SBUF-resident across the repeat
  loop (loads interleaved into rep 0's front phase for a fast first dispatch).
- All-(64,128)-tile attend loop: the two heads' score matmuls run as
  concurrent row-tiles {T0,T8}; AV is split into lo/hi half-contraction
  row-tiles accumulating into 4 separate PSUM banks (2 heads x lo/hi), summed
  during DVE evacuation. The jt loop never switches PE tiling mode (a
  (64,128)<->(128,128) switch costs ~140ns on HW; measured jt-step 771ns vs
  1051ns for the switching variant).
- proj bias folded into the DVE psum-evacuation add (no bias matmuls);
  x fp32->bf16 conversion on DVE (keeps ACT free for the exp stream).
PSUM: psS bufs=2 x 2 banks (scores/emissions/proj/transposes) + psAcc 4 x 1.
Attend is ACT-exp-paced (~1146ns/jt vs PE 771); deeper overlap is blocked by
the 16KB PSUM budget (see memory notes).
"""
import sys

sys.path.insert(0, "/opt/trn_rl_repo")

import numpy as np

N = 1024
D = 1024
H = 16
HD = 64
SCALE = HD ** -0.5
P = 128
NT = N // P
DTn = D // P
HC = 512
VW = 65

_CACHE: dict = {}


def _build_nc(repeat=1):
    import concourse.bass as bass
    import concourse.tile as tile
    from concourse import bacc, mybir
    from concourse.masks import make_identity
    from contextlib import ExitStack

    fp32 = mybir.dt.float32
    bf16 = mybir.dt.bfloat16
    EXP = mybir.ActivationFunctionType.Exp

    nc = bacc.Bacc("TRN2", debug=False, num_devices=8)
    x_d = nc.dram_tensor("x", [N, D], fp32, kind="ExternalInput").ap()
    wqkv_d = nc.dram_tensor("w_qkv", [D, 3 * D], fp32, kind="ExternalInput").ap()
    wproj_d = nc.dram_tensor("w_proj", [D, D], fp32, kind="ExternalInput").ap()
    b_d = nc.dram_tensor("b_proj", [1, D], fp32, kind="ExternalInput").ap()
    out_d = nc.dram_tensor("out", [N, D], fp32, kind="ExternalOutput").ap()

    with tile.TileContext(nc) as tc, ExitStack() as ctx:
        stg = ctx.enter_context(tc.tile_pool(name="stg", bufs=3))
        xbfp = ctx.enter_context(tc.tile_pool(name="xbf", bufs=2))
        xTp = ctx.enter_context(tc.tile_pool(name="xT", bufs=NT))
        wqp = ctx.enter_context(tc.tile_pool(name="wq", bufs=DTn))
        wkp = ctx.enter_context(tc.tile_pool(name="wk", bufs=DTn))
        wvp = ctx.enter_context(tc.tile_pool(name="wv", bufs=DTn))
        wprojp = ctx.enter_context(tc.tile_pool(name="wproj", bufs=DTn))
        qkTp = ctx.enter_context(tc.tile_pool(name="qkT", bufs=2 * NT))
        vp = ctx.enter_context(tc.tile_pool(name="vsb", bufs=NT))
        eTp = ctx.enter_context(tc.tile_pool(name="eT", bufs=5))
        attnp = ctx.enter_context(tc.tile_pool(name="attnT", bufs=2 * NT))
        avsbp = ctx.enter_context(tc.tile_pool(name="avsb", bufs=3))
        recp = ctx.enter_context(tc.tile_pool(name="rec", bufs=3))
        rbp = ctx.enter_context(tc.tile_pool(name="rb", bufs=3))
        constp = ctx.enter_context(tc.tile_pool(name="const", bufs=1))
        ysbp = ctx.enter_context(tc.tile_pool(name="ysb", bufs=2))
        psS = ctx.enter_context(tc.tile_pool(name="psS", bufs=2, space="PSUM"))
        psAcc = ctx.enter_context(tc.tile_pool(name="psAcc", bufs=4, space="PSUM"))

        # ---- loop-invariant: constants + all weights (SBUF-resident) ----
        b_stage = stg.tile([1, D], fp32, name="b_stage", tag="stg")
        nc.sync.dma_start(b_stage[:], b_d[:])
        b_full = constp.tile([P, D], fp32, name="b_full", tag="b_full")
        nc.gpsimd.partition_broadcast(b_full[:], b_stage[:])

        def load_w(dst_pool, src, src_col, name):
            tiles = []
            for t in range(DTn):
                s = stg.tile([P, 1024], fp32, name="stg_t", tag="stg")
                nc.sync.dma_start(
                    s[:], src[t * P:(t + 1) * P, src_col:src_col + 1024])
                w = dst_pool.tile([P, 1024], bf16, name=f"{name}{t}", tag=name)
                nc.vector.tensor_copy(w[:], s[:])
                tiles.append(w)
            return tiles

        # weights are loaded once, interleaved into rep 0's front phase for a
        # fast single-dispatch start; they stay SBUF-resident for later reps.
        wq = wk = wv = wproj = None

        for _rep in range(repeat):
            # --- x: load, convert (DVE), PE-transpose ---
            xT = [xTp.tile([P, N], bf16, name=f"xT{t}", tag="xT")
                  for t in range(DTn)]
            for it in range(NT):
                s = stg.tile([P, 1024], fp32, name="stg_t", tag="stg")
                nc.sync.dma_start(s[:], x_d[it * P:(it + 1) * P, :])
                xb = xbfp.tile([P, D], bf16, name="xb_t", tag="xb")
                nc.vector.tensor_copy(xb[:], s[:])
                for dt in range(DTn):
                    # SBUF->SBUF xbar transpose on the DMA engines: keeps the
                    # PE (and DVE evacuation copies) out of the front phase.
                    nc.sync.dma_start(xT[dt][:, it * P:(it + 1) * P],
                                      xb[:, dt * P:(dt + 1) * P],
                                      transpose=True)

            if _rep == 0:
                wq = load_w(wqp, wqkv_d, 0, "wq")
                wk = load_w(wkp, wqkv_d, 1024, "wk")

            qkT = [None] * (2 * NT)

            def emit_qkT(ft):
                wt = wq if ft < NT else wk
                col = (ft % NT) * P
                ps = psS.tile([P, N], fp32, name="pss_t", tag="pss")
                for dt in range(DTn):
                    for ic in range(2):
                        nc.tensor.matmul(
                            ps[:, ic * HC:(ic + 1) * HC],
                            lhsT=wt[dt][:, col:col + P],
                            rhs=xT[dt][:, ic * HC:(ic + 1) * HC],
                            start=(dt == 0), stop=(dt == DTn - 1))
                q = qkTp.tile([P, N], bf16, name=f"qkT{ft}", tag="qkT")
                nc.vector.tensor_copy(q[:], ps[:])
                qkT[ft] = q

            emit_qkT(0)
            emit_qkT(NT)

            if _rep == 0:
                wv = load_w(wvp, wqkv_d, 2048, "wv")

            vsb = [None] * NT

            def emit_v(jt):
                ps = psS.tile([P, N], fp32, name="pss_t", tag="pss")
                for dt in range(DTn):
                    for dc in range(2):
                        nc.tensor.matmul(
                            ps[:, dc * HC:(dc + 1) * HC],
                            lhsT=xT[dt][:, jt * P:(jt + 1) * P],
                            rhs=wv[dt][:, dc * HC:(dc + 1) * HC],
                            start=(dt == 0), stop=(dt == DTn - 1))
                v = vp.tile([P, H * VW], bf16, name=f"v{jt}", tag="v")
                nc.gpsimd.memset(v[:], 1.0)
                vv = v[:].rearrange("p (h c) -> p h c", c=VW)
                pv = ps[:].rearrange("p (h c) -> p h c", c=HD)
                nc.vector.tensor_copy(vv[:, :, 0:HD], pv)
                vsb[jt] = v

            emit_v(0)
            emit_v(1)

            attnT = [[attnp.tile([P, HC], bf16, name=f"attnT{c}_{q}",
                                 tag="attnT") for q in range(NT)]
                     for c in range(2)]

            def attend_pair(hp, icb, jt_hook=None):
                ha, hb = 2 * hp, 2 * hp + 1
                qa, ka = qkT[hp], qkT[NT + hp]
                i0 = icb * HC
                # lo/hi half-contraction accumulators: [head][half]
                av = {h: [psAcc.tile([VW, HC], fp32, name=f"av{h}_{icb}_{hf}",
                                     tag="av") for hf in range(2)]
                      for h in (ha, hb)}
                for jt in range(NT):
                    ps = psS.tile([P, N], fp32, name="pss_t", tag="pss")
                    nc.tensor.matmul(
                        ps[:, 0:HC],
                        lhsT=ka[0:HD, jt * P:(jt + 1) * P],
                        rhs=qa[0:HD, i0:i0 + HC],
                        start=True, stop=True)
                    nc.tensor.matmul(
                        ps[:, HC:N],
                        lhsT=ka[HD:P, jt * P:(jt + 1) * P],
                        rhs=qa[HD:P, i0:i0 + HC],
                        start=True, stop=True)
                    e = eTp.tile([P, N], bf16, name=f"e{hp}_{jt}", tag="e")
                    nc.scalar.activation(e[:], ps[:], EXP, scale=SCALE)
                    if jt_hook is not None:
                        jt_hook(jt)
                    st, sp = (jt == 0), (jt == NT - 1)
                    # T0/T8 alternating, each to its own PSUM bank
                    nc.tensor.matmul(
                        av[ha][0][:],
                        lhsT=vsb[jt][0:HD, ha * VW:(ha + 1) * VW],
                        rhs=e[0:HD, 0:HC], start=st, stop=sp)
                    nc.tensor.matmul(
                        av[ha][1][:],
                        lhsT=vsb[jt][HD:P, ha * VW:(ha + 1) * VW],
                        rhs=e[HD:P, 0:HC], start=st, stop=sp)
                    nc.tensor.matmul(
                        av[hb][0][:],
                        lhsT=vsb[jt][0:HD, hb * VW:(hb + 1) * VW],
                        rhs=e[0:HD, HC:N], start=st, stop=sp)
                    nc.tensor.matmul(
                        av[hb][1][:],
                        lhsT=vsb[jt][HD:P, hb * VW:(hb + 1) * VW],
                        rhs=e[HD:P, HC:N], start=st, stop=sp)
                for h, off in ((ha, 0), (hb, HD)):
                    # DVE cannot read two PSUM operands in one tensor_tensor:
                    # stage lo in SBUF, then add hi from PSUM.
                    avl = avsbp.tile([VW, HC], fp32, name=f"avl{h}", tag="avl",
                                      bufs=2)
                    nc.vector.tensor_copy(avl[:], av[h][0][:])
                    avs = avsbp.tile([VW, HC], bf16, name=f"avs{h}", tag="avs")
                    nc.vector.tensor_add(avs[:], avl[:], av[h][1][:])
                    recf = recp.tile([1, HC], fp32, name=f"recf{h}", tag="recf")
                    nc.vector.reciprocal(recf[:], avs[HD:VW, :])
                    rb = rbp.tile([HD, HC], fp32, name=f"rb{h}", tag="rb")
                    nc.gpsimd.partition_broadcast(rb[:], recf[:])
                    nc.vector.tensor_mul(attnT[icb][hp][off:off + HD, :],
                                         avs[0:HD, :], rb[:])

            def proj_tile(it):
                icb = it // 4
                ps = psS.tile([P, N], fp32, name="pss_t", tag="pss")
                for fc in range(2):
                    for dt in range(DTn):
                        nc.tensor.matmul(
                            ps[:, fc * HC:(fc + 1) * HC],
                            lhsT=attnT[icb][dt][:, (it % 4) * P:
                                                (it % 4 + 1) * P],
                            rhs=wproj[dt][:, fc * HC:(fc + 1) * HC],
                            start=(dt == 0), stop=(dt == DTn - 1))
                y = ysbp.tile([P, N], fp32, name="y_t", tag="y")
                nc.vector.tensor_add(y[:], ps[:], b_full[:])
                nc.sync.dma_start(out_d[it * P:(it + 1) * P, :], y[:])

            # pair 0 chunk 0: V(2..7) emitted inside the jt loop, one ahead
            def v_hook(jt):
                if jt + 2 < NT and vsb[jt + 2] is None:
                    emit_v(jt + 2)

            attend_pair(0, 0, jt_hook=v_hook)

            if _rep == 0:
                wproj = load_w(wprojp, wproj_d, 0, "wproj")

            for hp in range(1, NT):
                if qkT[hp] is None:
                    emit_qkT(hp)
                    emit_qkT(NT + hp)
                if hp + 1 < NT and qkT[hp + 1] is None:
                    emit_qkT(hp + 1)
                    emit_qkT(NT + hp + 1)
                attend_pair(hp, 0)
            for hp in range(NT):
                attend_pair(hp, 1)
                if hp >= 4:
                    proj_tile(hp - 4)
            for it in range(4, NT):
                proj_tile(it)

    nc.compile()
    return nc


def get_nc():
    if "nc" not in _CACHE:
        _CACHE["nc"] = _build_nc()
    return _CACHE["nc"]


def kernel(x, w_qkv, w_proj, b_proj):
    from concourse import bass_utils

    nc = get_nc()
    x = np.ascontiguousarray(x, dtype=np.float32)
    w_qkv = np.ascontiguousarray(w_qkv, dtype=np.float32)
    w_proj = np.ascontiguousarray(w_proj, dtype=np.float32)
    b2 = np.ascontiguousarray(b_proj, dtype=np.float32).reshape(1, D)
    in_maps = [
        {"x": x[i], "w_qkv": w_qkv, "w_proj": w_proj, "b_proj": b2}
        for i in range(8)
    ]
    res = bass_utils.run_bass_kernel_spmd(nc, in_maps, core_ids=list(range(8)))
    return np.stack([res.results[i]["out"] for i in range(8)], axis=0)


# revision 11
# speedup vs baseline: 1.3597x; 1.3597x over previous
"""v13.3: data-parallel attention (1 batch element per core, no collectives).

vs v11 baseline (295,868 ns -> ~242,000 ns best measured):
- Weights/constants loaded once and kept SBUF-resident across the repeat
  loop (loads interleaved into rep 0's front phase for a fast first dispatch).
- All-(64,128)-tile attend loop: the two heads' score matmuls run as
  concurrent row-tiles {T0,T8}; AV is split into lo/hi half-contraction
  row-tiles accumulating into 4 separate PSUM banks (2 heads x lo/hi), summed
  during DVE evacuation. The jt loop never switches PE tiling mode (a
  (64,128)<->(128,128) switch costs ~140ns on HW; measured jt-step 771ns vs
  1051ns for the switching variant).
- proj bias folded into the DVE psum-evacuation add (no bias matmuls);
  x fp32->bf16 conversion on DVE (keeps ACT free for the exp stream).
PSUM: psS bufs=2 x 2 banks (scores/emissions/proj/transposes) + psAcc 4 x 1.
Attend is ACT-exp-paced (~1146ns/jt vs PE 771); deeper overlap is blocked by
the 16KB PSUM budget (see memory notes).
"""
import sys

sys.path.insert(0, "/opt/trn_rl_repo")

import numpy as np

N = 1024
D = 1024
H = 16
HD = 64
SCALE = HD ** -0.5
P = 128
NT = N // P
DTn = D // P
HC = 512
VW = 65

_CACHE: dict = {}


def _build_nc(repeat=1):
    import concourse.bass as bass
    import concourse.tile as tile
    from concourse import bacc, mybir
    from concourse.masks import make_identity
    from contextlib import ExitStack

    fp32 = mybir.dt.float32
    bf16 = mybir.dt.bfloat16
    EXP = mybir.ActivationFunctionType.Exp

    nc = bacc.Bacc("TRN2", debug=False, num_devices=8)
    x_d = nc.dram_tensor("x", [N, D], fp32, kind="ExternalInput").ap()
    wqkv_d = nc.dram_tensor("w_qkv", [D, 3 * D], fp32, kind="ExternalInput").ap()
    wproj_d = nc.dram_tensor("w_proj", [D, D], fp32, kind="ExternalInput").ap()
    b_d = nc.dram_tensor("b_proj", [1, D], fp32, kind="ExternalInput").ap()
    out_d = nc.dram_tensor("out", [N, D], fp32, kind="ExternalOutput").ap()

    with tile.TileContext(nc) as tc, ExitStack() as ctx:
        stg = ctx.enter_context(tc.tile_pool(name="stg", bufs=3))
        xbfp = ctx.enter_context(tc.tile_pool(name="xbf", bufs=2))
        xTp = ctx.enter_context(tc.tile_pool(name="xT", bufs=NT))
        wqp = ctx.enter_context(tc.tile_pool(name="wq", bufs=DTn))
        wkp = ctx.enter_context(tc.tile_pool(name="wk", bufs=DTn))
        wvp = ctx.enter_context(tc.tile_pool(name="wv", bufs=DTn))
        wprojp = ctx.enter_context(tc.tile_pool(name="wproj", bufs=DTn))
        qkTp = ctx.enter_context(tc.tile_pool(name="qkT", bufs=2 * NT))
        vp = ctx.enter_context(tc.tile_pool(name="vsb", bufs=NT))
        eTp = ctx.enter_context(tc.tile_pool(name="eT", bufs=5))
        attnp = ctx.enter_context(tc.tile_pool(name="attnT", bufs=2 * NT))
        avsbp = ctx.enter_context(tc.tile_pool(name="avsb", bufs=3))
        recp = ctx.enter_context(tc.tile_pool(name="rec", bufs=3))
        rbp = ctx.enter_context(tc.tile_pool(name="rb", bufs=3))
        constp = ctx.enter_context(tc.tile_pool(name="const", bufs=1))
        ysbp = ctx.enter_context(tc.tile_pool(name="ysb", bufs=2))
        psS = ctx.enter_context(tc.tile_pool(name="psS", bufs=2, space="PSUM"))
        psAcc = ctx.enter_context(tc.tile_pool(name="psAcc", bufs=4, space="PSUM"))

        # ---- loop-invariant: constants + all weights (SBUF-resident) ----
        ident = constp.tile([P, P], bf16, name="ident", tag="ident")
        make_identity(nc, ident[:])
        b_stage = stg.tile([1, D], fp32, name="b_stage", tag="stg")
        nc.sync.dma_start(b_stage[:], b_d[:])
        b_full = constp.tile([P, D], fp32, name="b_full", tag="b_full")
        nc.gpsimd.partition_broadcast(b_full[:], b_stage[:])

        def load_w(dst_pool, src, src_col, name):
            tiles = []
            for t in range(DTn):
                s = stg.tile([P, 1024], fp32, name="stg_t", tag="stg")
                nc.sync.dma_start(
                    s[:], src[t * P:(t + 1) * P, src_col:src_col + 1024])
                w = dst_pool.tile([P, 1024], bf16, name=f"{name}{t}", tag=name)
                nc.vector.tensor_copy(w[:], s[:])
                tiles.append(w)
            return tiles

        # weights are loaded once, interleaved into rep 0's front phase for a
        # fast single-dispatch start; they stay SBUF-resident for later reps.
        wq = wk = wv = wproj = None

        for _rep in range(repeat):
            # --- x: load, convert (DVE), PE-transpose ---
            xT = [xTp.tile([P, N], bf16, name=f"xT{t}", tag="xT")
                  for t in range(DTn)]
            for it in range(NT):
                s = stg.tile([P, 1024], fp32, name="stg_t", tag="stg")
                nc.sync.dma_start(s[:], x_d[it * P:(it + 1) * P, :])
                xb = xbfp.tile([P, D], bf16, name="xb_t", tag="xb")
                nc.vector.tensor_copy(xb[:], s[:])
                for dt in range(DTn):
                    pt = psS.tile([P, P], bf16, name="pst_t", tag="pss")
                    nc.tensor.transpose(pt[:], xb[:, dt * P:(dt + 1) * P],
                                        ident[:])
                    nc.vector.tensor_copy(xT[dt][:, it * P:(it + 1) * P], pt[:])

            if _rep == 0:
                wq = load_w(wqp, wqkv_d, 0, "wq")
                wk = load_w(wkp, wqkv_d, 1024, "wk")

            qkT = [None] * (2 * NT)

            def emit_qkT(ft):
                wt = wq if ft < NT else wk
                col = (ft % NT) * P
                ps = psS.tile([P, N], fp32, name="pss_t", tag="pss")
                for dt in range(DTn):
                    for ic in range(2):
                        nc.tensor.matmul(
                            ps[:, ic * HC:(ic + 1) * HC],
                            lhsT=wt[dt][:, col:col + P],
                            rhs=xT[dt][:, ic * HC:(ic + 1) * HC],
                            start=(dt == 0), stop=(dt == DTn - 1))
                q = qkTp.tile([P, N], bf16, name=f"qkT{ft}", tag="qkT")
                nc.vector.tensor_copy(q[:], ps[:])
                qkT[ft] = q

            emit_qkT(0)
            emit_qkT(NT)

            if _rep == 0:
                wv = load_w(wvp, wqkv_d, 2048, "wv")

            vsb = [None] * NT

            def emit_v(jt):
                ps = psS.tile([P, N], fp32, name="pss_t", tag="pss")
                for dt in range(DTn):
                    for dc in range(2):
                        nc.tensor.matmul(
                            ps[:, dc * HC:(dc + 1) * HC],
                            lhsT=xT[dt][:, jt * P:(jt + 1) * P],
                            rhs=wv[dt][:, dc * HC:(dc + 1) * HC],
                            start=(dt == 0), stop=(dt == DTn - 1))
                v = vp.tile([P, H * VW], bf16, name=f"v{jt}", tag="v")
                nc.gpsimd.memset(v[:], 1.0)
                vv = v[:].rearrange("p (h c) -> p h c", c=VW)
                pv = ps[:].rearrange("p (h c) -> p h c", c=HD)
                nc.vector.tensor_copy(vv[:, :, 0:HD], pv)
                vsb[jt] = v

            emit_v(0)
            emit_v(1)

            attnT = [[attnp.tile([P, HC], bf16, name=f"attnT{c}_{q}",
                                 tag="attnT") for q in range(NT)]
                     for c in range(2)]

            def attend_pair(hp, icb, jt_hook=None):
                ha, hb = 2 * hp, 2 * hp + 1
                qa, ka = qkT[hp], qkT[NT + hp]
                i0 = icb * HC
                # lo/hi half-contraction accumulators: [head][half]
                av = {h: [psAcc.tile([VW, HC], fp32, name=f"av{h}_{icb}_{hf}",
                                     tag="av") for hf in range(2)]
                      for h in (ha, hb)}
                for jt in range(NT):
                    ps = psS.tile([P, N], fp32, name="pss_t", tag="pss")
                    nc.tensor.matmul(
                        ps[:, 0:HC],
                        lhsT=ka[0:HD, jt * P:(jt + 1) * P],
                        rhs=qa[0:HD, i0:i0 + HC],
                        start=True, stop=True)
                    nc.tensor.matmul(
                        ps[:, HC:N],
                        lhsT=ka[HD:P, jt * P:(jt + 1) * P],
                        rhs=qa[HD:P, i0:i0 + HC],
                        start=True, stop=True)
                    e = eTp.tile([P, N], bf16, name=f"e{hp}_{jt}", tag="e")
                    nc.scalar.activation(e[:], ps[:], EXP, scale=SCALE)
                    if jt_hook is not None:
                        jt_hook(jt)
                    st, sp = (jt == 0), (jt == NT - 1)
                    # T0/T8 alternating, each to its own PSUM bank
                    nc.tensor.matmul(
                        av[ha][0][:],
                        lhsT=vsb[jt][0:HD, ha * VW:(ha + 1) * VW],
                        rhs=e[0:HD, 0:HC], start=st, stop=sp)
                    nc.tensor.matmul(
                        av[ha][1][:],
                        lhsT=vsb[jt][HD:P, ha * VW:(ha + 1) * VW],
                        rhs=e[HD:P, 0:HC], start=st, stop=sp)
                    nc.tensor.matmul(
                        av[hb][0][:],
                        lhsT=vsb[jt][0:HD, hb * VW:(hb + 1) * VW],
                        rhs=e[0:HD, HC:N], start=st, stop=sp)
                    nc.tensor.matmul(
                        av[hb][1][:],
                        lhsT=vsb[jt][HD:P, hb * VW:(hb + 1) * VW],
                        rhs=e[HD:P, HC:N], start=st, stop=sp)
                for h, off in ((ha, 0), (hb, HD)):
                    # DVE cannot read two PSUM operands in one tensor_tensor:
                    # stage lo in SBUF, then add hi from PSUM.
                    avl = avsbp.tile([VW, HC], fp32, name=f"avl{h}", tag="avl",
                                      bufs=2)
                    nc.vector.tensor_copy(avl[:], av[h][0][:])
                    avs = avsbp.tile([VW, HC], bf16, name=f"avs{h}", tag="avs")
                    nc.vector.tensor_add(avs[:], avl[:], av[h][1][:])
                    recf = recp.tile([1, HC], fp32, name=f"recf{h}", tag="recf")
                    nc.vector.reciprocal(recf[:], avs[HD:VW, :])
                    rb = rbp.tile([HD, HC], fp32, name=f"rb{h}", tag="rb")
                    nc.gpsimd.partition_broadcast(rb[:], recf[:])
                    nc.vector.tensor_mul(attnT[icb][hp][off:off + HD, :],
                                         avs[0:HD, :], rb[:])

            def proj_tile(it):
                icb = it // 4
                ps = psS.tile([P, N], fp32, name="pss_t", tag="pss")
                for fc in range(2):
                    for dt in range(DTn):
                        nc.tensor.matmul(
                            ps[:, fc * HC:(fc + 1) * HC],
                            lhsT=attnT[icb][dt][:, (it % 4) * P:
                                                (it % 4 + 1) * P],
                            rhs=wproj[dt][:, fc * HC:(fc + 1) * HC],
                            start=(dt == 0), stop=(dt == DTn - 1))
                y = ysbp.tile([P, N], fp32, name="y_t", tag="y")
                nc.vector.tensor_add(y[:], ps[:], b_full[:])
                nc.sync.dma_start(out_d[it * P:(it + 1) * P, :], y[:])

            # pair 0 chunk 0: V(2..7) emitted inside the jt loop, one ahead
            def v_hook(jt):
                if jt + 2 < NT and vsb[jt + 2] is None:
                    emit_v(jt + 2)

            attend_pair(0, 0, jt_hook=v_hook)

            if _rep == 0:
                wproj = load_w(wprojp, wproj_d, 0, "wproj")

            for hp in range(1, NT):
                if qkT[hp] is None:
                    emit_qkT(hp)
                    emit_qkT(NT + hp)
                if hp + 1 < NT and qkT[hp + 1] is None:
                    emit_qkT(hp + 1)
                    emit_qkT(NT + hp + 1)
                attend_pair(hp, 0)
            for hp in range(NT):
                attend_pair(hp, 1)
                if hp >= 4:
                    proj_tile(hp - 4)
            for it in range(4, NT):
                proj_tile(it)

    nc.compile()
    return nc


def get_nc():
    if "nc" not in _CACHE:
        _CACHE["nc"] = _build_nc()
    return _CACHE["nc"]


def kernel(x, w_qkv, w_proj, b_proj):
    from concourse import bass_utils

    nc = get_nc()
    x = np.ascontiguousarray(x, dtype=np.float32)
    w_qkv = np.ascontiguousarray(w_qkv, dtype=np.float32)
    w_proj = np.ascontiguousarray(w_proj, dtype=np.float32)
    b2 = np.ascontiguousarray(b_proj, dtype=np.float32).reshape(1, D)
    in_maps = [
        {"x": x[i], "w_qkv": w_qkv, "w_proj": w_proj, "b_proj": b2}
        for i in range(8)
    ]
    res = bass_utils.run_bass_kernel_spmd(nc, in_maps, core_ids=list(range(8)))
    return np.stack([res.results[i]["out"] for i in range(8)], axis=0)
